# revision 51
# baseline (speedup 1.0000x reference)
"""Trainium2 Bass kernel for BilinearInteraction.

out[b, p] = x[b, i_p, :] @ W[p] @ x[b, j_p, :]  for the 780 field pairs
(i, j), i < j, of F=40 fields (row-major triu order).

8 NeuronCores, data-parallel over batch (B_loc=256). All arithmetic is
fp16 with fp32 accumulation (rel err ~6e-4 vs the 2e-2 gate).

Four-engine balanced pipeline; pairs split into two layout families:

 - PAIR-MAJOR (pairs with i >= BM_IMAX, as 2-pair tiles):
     stage 1  PE   : Y[(2 pairs, e), b] = Wtile.T @ xT_i   (N=256/tile)
     mult         : per-group pattern (PM_PATTERN, "AABCAB"):
                    A: DVE tensor_tensor fused from PSUM (1x, no evict
                       -- also recycles the Y PSUM tile fastest)
                    B: ACT evict fp16 -> DVE 2x packed mult
                    C: ACT evict fp16 -> GPSIMD tensor_tensor mult
     reduce   PE   : sliding-window mask matmuls (M=128): 64 tiles
                    accumulate into one [128,256] PSUM region, so only
                    ceil(NTILES/64) bank evicts + DMAs remain.
 - BATCH-MAJOR (pairs with i < BM_IMAX, j-runs per i):
     stage 1  PE   : Y[b, (j, e)] = xT_i.T @ Wbm  (x-stationary, N=512)
     mult    POOL  : GPSIMD tensor_tensor after ACT evict
     reduce   DVE  : binary tree-add over the 64-wide e segments --
                    fp16 tensor_tensor adds run at the DVE 2x packed
                    rate (tensor_reduce / scan have no fast mode).
   Batch-major output lands directly in natural [b, pair] layout.

Other scheduling ingredients (all tuned against TimelineSim, which is
what the harness reports as HW exec time; correctness runs on real
TRN2 via PJRT):
 - Wt3 streamed in 55-tile chunks through a rotating 3-buffer pool,
   prefetched 2 chunks ahead; x tensors DMA'd in first-use order.
 - PSUM budget (8 banks): 3x 2-bank Y-groups + 1 batch-major block
   + 1 reduce accumulator.
 - z-tile pools deeper than the reduce lag (ZPM_BUFS > REDUCE_DELAY)
   so delayed PE reduces never block the multiply stage.
 - 30 tiny warm-up matmuls ramp the PE p-state during the DMA-bound
   startup window.

TimelineSim: 97.9 us vs 119.4 us for the previous 2-engine kernel
(PE+ACT bound); engine busy ~80/82/69/70 us (PE/DVE/ACT/POOL).
"""

import numpy as np
import ml_dtypes  # noqa: F401  (kept for environments where fp8 tests run)

import concourse.bass as bass
import concourse.mybir as mybir
import concourse.tile as tile
from concourse import bacc
from concourse.bass_utils import run_bass_kernel_spmd

B, F, D = 2048, 40, 64
P = F * (F - 1) // 2  # 780
NCORES = 8
B_LOC = B // NCORES  # 256
F32 = mybir.dt.float32
F16 = mybir.dt.float16

NCHUNK = F // 2  # 20 xT chunks (2 fields each)

# ---- tunable configuration (applied by _derive) ----
CFG = dict(
    BM_IMAX=4,      # pairs with i < BM_IMAX go batch-major
    RB_HIPRI=30,    # schedule rb evict with raised priority (offset)
    GMAX=4,         # pair-major tiles per PSUM Y-group
    YPM_BUFS=3,     # pair-major Y-group buffers (PSUM)
    BM_BLK=8,       # batch-major pairs per stage-1 matmul
    YBM_BUFS=1,     # batch-major Y-block buffers (PSUM)
    RPM_BUFS=1,     # reduce-accumulator buffers (PSUM)
    # pair-major group mult-path pattern, cycled per group:
    #  'A' = fused DVE mult from PSUM (1x, no evict)
    #  'B' = ACT evict -> DVE 2x mult
    #  'C' = ACT evict -> GPSIMD (Pool) mult
    PM_PATTERN="AABCAB",
    REDUCE_DELAY=10,  # groups of reduce-matmul lag
    POOL_DELAY=2,    # runs of scan-reduce lag
    BM_FUSED_NUM=0,  # bm runs with fused-DVE mult (of BM_FUSED_DEN)
    BM_FUSED_DEN=2,
    BM_MULT="P",     # bm mult engine pattern per block: V=DVE 2x, P=POOL
    BM_RED="tree",   # bm reduce: "tree" (DVE 2x adds) or "scan"
    ZPM_BUFS=13,
    YBF_BUFS=3,
    ZBM_BUFS=5,
    BM_EVERY=0,      # 0 = auto
    BM_START=9,      # delay first bm block by this many steps
    BM_PACK=0,       # if >0: emit bm blocks every BM_PACK steps until done
    POOL_SPLIT=1,    # split C-group pool mults into this many instrs
    PE_WARM=30,      # dummy warmup matmuls at t=0 (N cols each=64)
    RB_EVICT="A",    # rb evict engine per bank: V=DVE, A=ACT
    SKIP_DMA=False,  # diagnostic: emit no input DMAs
    TINY_DMA=False,  # diagnostic: input DMAs transfer only 64 cols
    ONLY="",         # diagnostic: "PM" or "BM" family only
)

TILES_PER_BANK = 64  # M=128 reduce: 64 tiles -> one [128, 256] region


def _derive():
    global BM_IMAX, GMAX, TILES, NTILES, NBANKS, OUT_ROWS, PM_PATTERN
    global REDUCE_DELAY, POOL_DELAY, BM_I, BM_N, BMP, BM_OFF, BM_BLK
    global BM_FUSED_NUM, BM_FUSED_DEN, GROUPS
    BM_IMAX = CFG["BM_IMAX"]
    GMAX = CFG["GMAX"]
    PM_PATTERN = CFG["PM_PATTERN"]
    REDUCE_DELAY = CFG["REDUCE_DELAY"]
    POOL_DELAY = CFG["POOL_DELAY"]
    BM_BLK = CFG["BM_BLK"]
    BM_FUSED_NUM = CFG["BM_FUSED_NUM"]
    BM_FUSED_DEN = CFG["BM_FUSED_DEN"]
    TILES = [(t, i) for t in range(NCHUNK) for i in range(BM_IMAX, 2 * t + 1)]
    NTILES = len(TILES)
    NBANKS = (NTILES + TILES_PER_BANK - 1) // TILES_PER_BANK
    OUT_ROWS = NBANKS * 128
    BM_I = list(range(BM_IMAX))
    BM_N = [F - 1 - i for i in BM_I]
    BMP = sum(BM_N)
    BM_OFF = np.concatenate([[0], np.cumsum(BM_N)]).astype(int)
    groups = []
    k = 0
    for t in range(NCHUNK):
        ilist = list(range(BM_IMAX, 2 * t + 1))
        while ilist:
            take = min(GMAX, len(ilist), TILES_PER_BANK - (k % TILES_PER_BANK))
            groups.append((t, ilist[:take]))
            ilist = ilist[take:]
            k += take
    GROUPS = groups


_derive()


def host_prep(W: np.ndarray):
    """Build Wt3 (pair-major lhsT), Wbm (batch-major rhs), the sliding
    mask window, and output permutation info."""
    Wt2 = np.ascontiguousarray(W.transpose(1, 0, 2))  # [64, 780, 64]
    pair_idx = -np.ones((F, F), dtype=np.int64)
    k = 0
    for i in range(F):
        for j in range(i + 1, F):
            pair_idx[i, j] = k
            k += 1

    # pair-major lhsT: per tile a [64, 128] block = [W_pA | W_pB] columns
    Wt3 = np.zeros((D, NTILES * 128), dtype=np.float32)
    rows = []
    for k2, (t, i) in enumerate(TILES):
        jA, jB = 2 * t, 2 * t + 1
        pA = pair_idx[i, jA] if i < jA else -1
        pB = pair_idx[i, jB]
        if pA >= 0:
            Wt3[:, k2 * 128 : k2 * 128 + 64] = Wt2[:, pA, :]
        Wt3[:, k2 * 128 + 64 : k2 * 128 + 128] = Wt2[:, pB, :]
        rows.append((pA, pB))

    # batch-major rhs: Wbm[d, (pair, e)] for pairs (i, j), i < BM_IMAX
    Wbm = np.zeros((D, BMP * 64), dtype=np.float32)
    bm_pids = []
    c = 0
    for i in BM_I:
        for j in range(i + 1, F):
            p = pair_idx[i, j]
            Wbm[:, c * 64 : (c + 1) * 64] = W[p]  # [d, e]
            bm_pids.append(p)
            c += 1

    # sliding mask window [128, 512]: maskwin[k, c] = 1 iff
    # (k < 64 and c == 256) or (k >= 64 and c == 257).
    # reduce slot s uses lhsT = maskwin[:, 256 - 2s : 384 - 2s].
    maskwin = np.zeros((128, 512), dtype=np.float32)
    maskwin[0:64, 256] = 1.0
    maskwin[64:128, 257] = 1.0

    # out row of tile k: bank = k//64, s = k%64 -> rows bank*128 + 2s (+1)
    perm_src = np.zeros(P, dtype=np.int64)
    pm_mask = np.zeros(P, dtype=bool)
    for k2, (pA, pB) in enumerate(rows):
        bank, s = divmod(k2, TILES_PER_BANK)
        rowA = bank * 128 + 2 * s
        if pA >= 0:
            perm_src[pA] = rowA
            pm_mask[pA] = True
        perm_src[pB] = rowA + 1
        pm_mask[pB] = True

    segmask = np.ones((128, 2496), dtype=np.float16)
    segmask[:, 0::64] = 0.0

    return (
        Wt3.astype(np.float16),
        Wbm.astype(np.float16),
        maskwin.astype(np.float16),
        segmask,
        perm_src,
        pm_mask,
        np.array(bm_pids, dtype=np.int64),
    )


def build_nc():
    nc = bacc.Bacc("TRN2", target_bir_lowering=False, debug=False)

    xtlo_dram = nc.dram_tensor("xtlo", [64, F * B_LOC], F16, kind="ExternalInput").ap()
    xtc_dram = nc.dram_tensor(
        "xtc", [128, NCHUNK * B_LOC], F16, kind="ExternalInput"
    ).ap()
    xnat_dram = nc.dram_tensor(
        "xnat", [128, 2 * F * D], F16, kind="ExternalInput"
    ).ap()
    wt_dram = nc.dram_tensor("Wt3", [D, NTILES * 128], F16, kind="ExternalInput").ap()
    wbm_dram = nc.dram_tensor("Wbm", [D, BMP * 64], F16, kind="ExternalInput").ap()
    mask_dram = nc.dram_tensor("maskwin", [128, 512], F16, kind="ExternalInput").ap()
    segmask_dram = nc.dram_tensor("segmask", [128, 2496], F16, kind="ExternalInput").ap()
    out_dram = nc.dram_tensor("outT", [OUT_ROWS, B_LOC], F32, kind="ExternalOutput").ap()
    outbm_dram = nc.dram_tensor("outbm", [2 * 128, BMP], F32, kind="ExternalOutput").ap()

    # W streaming: chunks from a rotating pool; small first chunks so
    # the first PE groups start as early as possible
    WT_CSZ = CFG.get("WT_CSZ", 55)
    wt_rng = []
    lo = 0
    for sz in CFG.get("WT_FIRST", ()):
        if lo < NTILES:
            wt_rng.append((lo, min(lo + sz, NTILES)))
            lo = min(lo + sz, NTILES)
    while lo < NTILES:
        wt_rng.append((lo, min(lo + WT_CSZ, NTILES)))
        lo = min(lo + WT_CSZ, NTILES)
    WT_CHUNKS = len(wt_rng)
    # wbm chunks on i-run boundaries (2 chunks)
    wbm_i_splits = [0, (BM_IMAX + 1) // 2, BM_IMAX]
    WBM_CHUNKS = len(wbm_i_splits) - 1
    wbm_rng = [
        (int(BM_OFF[wbm_i_splits[c]]), int(BM_OFF[wbm_i_splits[c + 1]]))
        for c in range(WBM_CHUNKS)
    ]

    with tile.TileContext(nc) as tc:
        with (
            tc.tile_pool(name="persist", bufs=1) as persist,
            tc.tile_pool(name="zpm", bufs=CFG["ZPM_BUFS"]) as zpm_pool,
            tc.tile_pool(name="ybf", bufs=CFG["YBF_BUFS"]) as ybf_pool,
            tc.tile_pool(name="wtc", bufs=CFG.get("WTC_BUFS", 3)) as wtc_pool,
            tc.tile_pool(name="zbm", bufs=CFG["ZBM_BUFS"]) as zbm_pool,
            tc.tile_pool(name="scan", bufs=2) as scan_pool,
            tc.tile_pool(name="ybmsb", bufs=2) as ybmsb_pool,
            tc.tile_pool(name="opm", bufs=2) as opm_pool,
            tc.tile_pool(
                name="ypm", bufs=CFG["YPM_BUFS"], space=bass.MemorySpace.PSUM
            ) as ypm_pool,
            tc.tile_pool(
                name="ybm", bufs=CFG["YBM_BUFS"], space=bass.MemorySpace.PSUM
            ) as ybm_pool,
            tc.tile_pool(
                name="rpm", bufs=CFG["RPM_BUFS"], space=bass.MemorySpace.PSUM
            ) as rpm_pool,
        ):
            def dma(out_ap, in_ap):
                if CFG["SKIP_DMA"]:
                    return
                if CFG["TINY_DMA"]:
                    out_ap = out_ap[:, :64]
                    in_ap = in_ap[:, :64]
                nc.sync.dma_start(out=out_ap, in_=in_ap)

            # issue order matters: the DMA device and HWDGE serialize, so
            # load exactly what the first compute steps need first, in
            # small pieces (xtlo by field range, in separate tiles so
            # consumers only wait on their own range's DMA).
            XTLO_FPC = CFG.get("XTLO_FPC", 40)  # fields per xtlo chunk
            xtlo_tiles = []
            n_xtlo = F // XTLO_FPC
            for c in range(n_xtlo):
                t_ = persist.tile(
                    [64, XTLO_FPC * B_LOC], F16, tag=f"xtlo{c}", name=f"xtlo{c}"
                )
                xtlo_tiles.append(t_)

            def xtlo_sl(f0, ncols):
                c = f0 // XTLO_FPC
                off = (f0 - c * XTLO_FPC) * B_LOC
                return xtlo_tiles[c][:, off : off + ncols]

            dma(xtlo_tiles[0][:], xtlo_dram[:, : XTLO_FPC * B_LOC])
            xtc = persist.tile([128, NCHUNK * B_LOC], F16, tag="xtc")
            nq = NCHUNK * B_LOC // 2

            wt_tiles = {}

            def ensure_wt(c):
                if c in wt_tiles or c >= WT_CHUNKS:
                    return
                lo, hi = wt_rng[c]
                wt_ = wtc_pool.tile(
                    [64, WT_CSZ * 128], F16, tag="wtc", name=f"wtc{c}"
                )
                dma(wt_[:, : (hi - lo) * 128], wt_dram[:, lo * 128 : hi * 128])
                wt_tiles[c] = wt_
            wbm_tiles = []
            for c in range(WBM_CHUNKS):
                lo, hi = wbm_rng[c]
                wt_ = persist.tile(
                    [64, (hi - lo) * 64], F16, tag=f"wbm{c}", name=f"wbm{c}"
                )
                wbm_tiles.append(wt_)
            xnat = persist.tile([128, 2 * F * D], F16, tag="xnat")
            segmask = persist.tile([128, 2496], F16, tag="segmask")
            with tc.high_priority():
                nc.vector.memset(segmask[:], 1.0)
                nc.vector.memset(
                    segmask[:].rearrange("p (n w) -> p n w", w=64)[:, :, 0:1],
                    0.0,
                )


            ensure_wt(0)
            dma(xtc[:, :nq], xtc_dram[:, :nq])
            if n_xtlo > 1:
                dma(
                    xtlo_tiles[1][:],
                    xtlo_dram[:, XTLO_FPC * B_LOC : 2 * XTLO_FPC * B_LOC],
                )
            maskwin = persist.tile([128, 512], F16, tag="maskwin")
            dma(maskwin[:], mask_dram[:])
            dma(
                wbm_tiles[0][:],
                wbm_dram[:, wbm_rng[0][0] * 64 : wbm_rng[0][1] * 64],
            )
            dma(xnat[:, : F * D], xnat_dram[:, : F * D])
            ensure_wt(1)
            if n_xtlo > 2:
                dma(
                    xtlo_tiles[2][:],
                    xtlo_dram[:, 2 * XTLO_FPC * B_LOC : 3 * XTLO_FPC * B_LOC],
                )
            dma(xtc[:, nq:], xtc_dram[:, nq:])
            dma(
                wbm_tiles[1][:],
                wbm_dram[:, wbm_rng[1][0] * 64 : wbm_rng[1][1] * 64],
            )
            dma(xnat[:, F * D :], xnat_dram[:, F * D :])
            if n_xtlo > 3:
                dma(
                    xtlo_tiles[3][:],
                    xtlo_dram[:, 3 * XTLO_FPC * B_LOC : 4 * XTLO_FPC * B_LOC],
                )

            strips = [
                persist.tile([128, BMP], F32, tag=f"strip{h}", name=f"strip{h}")
                for h in range(2)
            ]


            def wt_chunk_of(k):
                for c, (lo, hi) in enumerate(wt_rng):
                    if lo <= k < hi:
                        return c
                raise AssertionError

            def wt_slice(k):
                c = wt_chunk_of(k)
                ensure_wt(c + 1)
                ensure_wt(c + 2)
                lo = wt_rng[c][0]
                return wt_tiles[c][:, (k - lo) * 128 : (k - lo + 1) * 128]

            def wbm_slice(pc0, npair):
                for c in range(WBM_CHUNKS):
                    lo, hi = wbm_rng[c]
                    if lo <= pc0 and pc0 + npair <= hi:
                        return wbm_tiles[c][
                            :, (pc0 - lo) * 64 : (pc0 - lo + npair) * 64
                        ]
                raise AssertionError

            # ---------- batch-major runs ----------
            # run = (i, h), emitted one stage-1 block at a time so the
            # ACT/DVE queues see a smooth trickle instead of 5-block
            # bursts; the scan reduce fires POOL_DELAY runs later.
            bm_runs = [(i, h) for h in (0, 1) for i in BM_I]
            bm_blocks = []  # (run_idx, j0, nb, first, last)
            for ri, (i, h) in enumerate(bm_runs):
                n = F - 1 - i
                j0 = 0
                while j0 < n:
                    nb = min(BM_BLK, n - j0)
                    bm_blocks.append((ri, j0, nb, j0 == 0, j0 + nb == n))
                    j0 += nb
            bm_zrun = {}
            bm_done = {0: 0, 1: 0}
            bm_runs_per_h = {
                h: sum(1 for (_, hh) in bm_runs if hh == h) for h in (0, 1)
            }

            def emit_bm_block(blk_idx):
                ri, j0, nb, first, last = bm_blocks[blk_idx]
                i, h = bm_runs[ri]
                n = F - 1 - i
                pc0 = int(BM_OFF[i])
                fused = (ri % BM_FUSED_DEN) < BM_FUSED_NUM
                if first:
                    bm_zrun[ri] = zbm_pool.tile([128, 2496], F16, tag="zbm", name=f"zbm{ri}")
                zrun = bm_zrun[ri]
                lhsT = xtlo_sl(i, B_LOC)[:, h * 128 : h * 128 + 128]
                y = ybm_pool.tile([128, BM_BLK * 64], F32, tag="ybm", name="ybm")
                nc.tensor.matmul(
                    y[:, : nb * 64],
                    lhsT,
                    wbm_slice(pc0 + j0, nb),
                    start=True,
                    stop=True,
                )
                xsl = xnat[
                    :,
                    h * F * D + (i + 1 + j0) * 64 : h * F * D
                    + (i + 1 + j0 + nb) * 64,
                ]
                if fused:
                    nc.vector.tensor_tensor(
                        zrun[:, j0 * 64 : (j0 + nb) * 64],
                        y[:, : nb * 64],
                        xsl,
                        mybir.AluOpType.mult,
                    )
                else:
                    ybf = ybmsb_pool.tile([128, BM_BLK * 64], F16, tag="ybmsb")
                    nc.scalar.copy(out=ybf[:, : nb * 64], in_=y[:, : nb * 64])
                    bmp = CFG["BM_MULT"]
                    meng = (
                        nc.gpsimd if bmp[blk_idx % len(bmp)] == "P" else nc.vector
                    )
                    meng.tensor_tensor(
                        zrun[:, j0 * 64 : (j0 + nb) * 64],
                        ybf[:, : nb * 64],
                        xsl,
                        mybir.AluOpType.mult,
                    )
                if last:
                    return (zrun, i, h, n, pc0)
                return None

            def emit_bm_pool(st):
                if CFG["BM_RED"] == "scan":
                    # masked prefix-sum scan: state = mask*state + z; the
                    # value at column 64k+63 is the k-th pair's segment sum.
                    zrun, i, h, n, pc0 = st
                    sc = scan_pool.tile([128, 2496], F16, tag="scan", name="scan")
                    nc.vector.tensor_tensor_scan(
                        out=sc[:, : n * 64],
                        data0=segmask[:, : n * 64],
                        data1=zrun[:, : n * 64],
                        initial=0.0,
                        op0=mybir.AluOpType.mult,
                        op1=mybir.AluOpType.add,
                    )
                    nc.vector.tensor_copy(
                        out=strips[h][:, pc0 : pc0 + n],
                        in_=sc[:, : n * 64].rearrange("p (n w) -> p n w", w=64)[
                            :, :, 63:64
                        ],
                    )
                    return
                # binary tree-add over the 64-wide segments: fp16 adds run
                # at the DVE 2x packed rate (the scan has no fast mode).
                zrun, i, h, n, pc0 = st
                sc = scan_pool.tile([128, 2496], F16, tag="scan", name="scan")
                src_t, s_off = zrun, 0
                w = 64
                off = 0
                while w > 2:
                    half = w // 2
                    a = src_t[:, s_off : s_off + n * w].rearrange(
                        "p (n w) -> p n w", w=w
                    )
                    nc.vector.tensor_tensor(
                        sc[:, off : off + n * half].rearrange(
                            "p (n w) -> p n w", w=half
                        ),
                        a[:, :, :half],
                        a[:, :, half:],
                        mybir.AluOpType.add,
                    )
                    src_t, s_off = sc, off
                    off += n * half
                    w = half
                a = src_t[:, s_off : s_off + n * 2].rearrange(
                    "p (n w) -> p n w", w=2
                )
                nc.vector.tensor_tensor(
                    strips[h][:, pc0 : pc0 + n].rearrange("p (n o) -> p n o", o=1),
                    a[:, :, 0:1],
                    a[:, :, 1:2],
                    mybir.AluOpType.add,
                )
                bm_done[h] += 1
                if bm_done[h] == bm_runs_per_h[h]:
                    nc.sync.dma_start(
                        out=outbm_dram[h * 128 : (h + 1) * 128, :],
                        in_=strips[h][:],
                    )

            # ---------- pair-major ----------
            rbs = [None]

            def emit_reduce(z, k0, gsz):
                for idx in range(gsz):
                    kt = k0 + idx
                    bank, s = divmod(kt, TILES_PER_BANK)
                    if s == 0:
                        rbs[0] = rpm_pool.tile([128, B_LOC], F32, tag="rb", name="rb")
                    rb = rbs[0]
                    last = (s == TILES_PER_BANK - 1) or (kt == NTILES - 1)
                    nc.tensor.matmul(
                        rb[:, :],
                        maskwin[:, 256 - 2 * s : 384 - 2 * s],
                        z[:, idx * B_LOC : (idx + 1) * B_LOC],
                        start=(s == 0),
                        stop=last,
                        tile_position=(0, 0),
                        skip_group_check=True,
                    )
                    if last:
                        import contextlib

                        ob = opm_pool.tile([128, B_LOC], F32, tag="ob")
                        rbe = CFG["RB_EVICT"]
                        hp = (
                            tc.high_priority(offset=CFG["RB_HIPRI"])
                            if CFG["RB_HIPRI"]
                            else contextlib.nullcontext()
                        )
                        with hp:
                            if rbe[bank % len(rbe)] == "A":
                                nc.scalar.copy(out=ob[:], in_=rb[:, :])
                            else:
                                nc.vector.tensor_copy(out=ob[:], in_=rb[:, :])
                        nc.sync.dma_start(
                            out=out_dram[bank * 128 : (bank + 1) * 128, :],
                            in_=ob[:],
                        )

            def emit_pm_group(gidx, k):
                t, ilist = GROUPS[gidx]
                gsz = len(ilist)
                path = PM_PATTERN[gidx % len(PM_PATTERN)]
                y = ypm_pool.tile([128, GMAX * B_LOC], F32, tag="y")
                for idx, i in enumerate(ilist):
                    nc.tensor.matmul(
                        y[:, idx * B_LOC : (idx + 1) * B_LOC],
                        wt_slice(k + idx),
                        xtlo_sl(i, B_LOC),
                        start=True,
                        stop=True,
                    )
                z = zpm_pool.tile([128, GMAX * B_LOC], F16, tag="z")
                in1 = xtc[:, None, t * B_LOC : (t + 1) * B_LOC].to_broadcast(
                    [128, gsz, B_LOC]
                )
                zr = z[:, : gsz * B_LOC].rearrange("p (n b) -> p n b", n=gsz)
                if path == "A":
                    nc.vector.tensor_tensor(
                        zr,
                        y[:, : gsz * B_LOC].rearrange("p (n b) -> p n b", n=gsz),
                        in1,
                        mybir.AluOpType.mult,
                    )
                else:
                    ybf = ybf_pool.tile([128, GMAX * B_LOC], F16, tag="ybf")
                    if path == "D":
                        nc.vector.tensor_copy(
                            out=ybf[:, : gsz * B_LOC], in_=y[:, : gsz * B_LOC]
                        )
                    else:
                        nc.scalar.copy(
                            out=ybf[:, : gsz * B_LOC], in_=y[:, : gsz * B_LOC]
                        )
                    if path == "B":
                        ybr = ybf[:, : gsz * B_LOC].rearrange(
                            "p (n b) -> p n b", n=gsz
                        )
                        nc.vector.tensor_tensor(zr, ybr, in1, mybir.AluOpType.mult)
                    else:
                        # split pool mults: finer-grained POOL queue entries
                        nsp = CFG["POOL_SPLIT"]
                        per = (gsz + nsp - 1) // nsp
                        for s0 in range(0, gsz, per):
                            ss = min(per, gsz - s0)
                            nc.gpsimd.tensor_tensor(
                                z[:, s0 * B_LOC : (s0 + ss) * B_LOC].rearrange(
                                    "p (n b) -> p n b", n=ss
                                ),
                                ybf[:, s0 * B_LOC : (s0 + ss) * B_LOC].rearrange(
                                    "p (n b) -> p n b", n=ss
                                ),
                                xtc[
                                    :, None, t * B_LOC : (t + 1) * B_LOC
                                ].to_broadcast([128, ss, B_LOC]),
                                mybir.AluOpType.mult,
                            )
                return (z, k, gsz)

            if CFG["PE_WARM"]:
                wsrc = persist.tile([64, 128], F16, tag="warmsrc")
                with tc.high_priority():
                    nc.vector.memset(wsrc[:], 0.0)
                wdst = rpm_pool.tile([128, B_LOC], F32, tag="rb", name="warm")
                for _ in range(CFG["PE_WARM"]):
                    nc.tensor.matmul(
                        wdst[:, :64],
                        wsrc[:, :],
                        wsrc[:, :64],
                        start=True,
                        stop=True,
                    )

            # ---------- interleaved emission ----------
            npm = len(GROUPS) if CFG["ONLY"] != "BM" else 0
            nbm = len(bm_blocks) if CFG["ONLY"] != "PM" else 0
            pend_red = []
            pend_pool = []
            k = 0
            gi = 0
            bi = 0
            # spread bm BLOCKS evenly among pm groups
            bm_every = CFG["BM_EVERY"] or max(1, (npm + nbm) // max(nbm, 1))
            step = 0
            while gi < npm or bi < nbm:
                be = CFG["BM_PACK"] or bm_every
                do_bm = (
                    step >= CFG["BM_START"]
                    and step % be == be - 1
                    and bi < nbm
                ) or gi >= npm
                if do_bm:
                    st = emit_bm_block(bi)
                    bi += 1
                    if st is not None:
                        pend_pool.append(st)
                        if len(pend_pool) > POOL_DELAY:
                            emit_bm_pool(pend_pool.pop(0))
                else:
                    if CFG.get("RED_FIRST") and len(pend_red) > REDUCE_DELAY:
                        emit_reduce(*pend_red.pop(0))
                    st = emit_pm_group(gi, k)
                    k += st[2]
                    gi += 1
                    pend_red.append(st)
                    if not CFG.get("RED_FIRST") and len(pend_red) > REDUCE_DELAY:
                        emit_reduce(*pend_red.pop(0))
                step += 1
            while pend_red:
                emit_reduce(*pend_red.pop(0))
            while pend_pool:
                emit_bm_pool(pend_pool.pop(0))

            if nbm > 0 and CFG["BM_RED"] == "scan":
                for h in range(2):
                    nc.sync.dma_start(
                        out=outbm_dram[h * 128 : (h + 1) * 128, :],
                        in_=strips[h][:],
                    )

    nc.compile()
    return nc


_NC = None


def kernel(x: np.ndarray, W: np.ndarray) -> np.ndarray:
    global _NC
    x = np.ascontiguousarray(np.asarray(x, dtype=np.float32))
    W = np.ascontiguousarray(np.asarray(W, dtype=np.float32))
    assert x.shape == (B, F, D) and W.shape == (P, D, D)

    Wt3, Wbm, maskwin, segmask, perm_src, pm_mask, bm_pids = host_prep(W)

    if _NC is None:
        _NC = build_nc()

    in_maps = []
    for c in range(NCORES):
        xs = x[c * B_LOC : (c + 1) * B_LOC]  # [256, 40, 64]
        xtlo = np.ascontiguousarray(
            xs.transpose(2, 1, 0).reshape(D, F * B_LOC)
        ).astype(np.float16)
        v = xs.transpose(1, 2, 0).reshape(NCHUNK, 2, D, B_LOC)
        xtc = np.ascontiguousarray(
            v.transpose(1, 2, 0, 3).reshape(128, NCHUNK * B_LOC)
        ).astype(np.float16)
        xnat = np.ascontiguousarray(
            xs.reshape(2, 128, F * D).transpose(1, 0, 2).reshape(128, 2 * F * D)
        ).astype(np.float16)
        in_maps.append(
            {
                "xtlo": xtlo,
                "xtc": xtc,
                "xnat": xnat,
                "Wt3": Wt3,
                "Wbm": Wbm,
                "maskwin": maskwin,
                "segmask": segmask,
            }
        )
    res = run_bass_kernel_spmd(_NC, in_maps, core_ids=list(range(NCORES)))
    out = np.empty((B, P), dtype=np.float32)
    pm_pids = np.nonzero(pm_mask)[0]
    for c in range(NCORES):
        outT = res.results[c]["outT"]  # [OUT_ROWS, B_LOC]
        outbm = res.results[c]["outbm"]  # [256, BMP]
        bs = slice(c * B_LOC, (c + 1) * B_LOC)
        out[bs, :][:, pm_pids] = outT[perm_src[pm_pids], :].T
        out[bs, :][:, bm_pids] = outbm
    return out
